# revision 17
# baseline (speedup 1.0000x reference)
"""Sparse attention (template/search) Trainium2 kernel.

Model (per batch b):
  qkv = x @ qkv_w.T                  -> split to q, k, v heads (12 heads, hd=64)
  template tokens   [0, 256)  attend to template keys only
  search   tokens [256, 1280) attend to all 1280 keys
  out = softmax(q k^T / 8) v   per head, concat heads, @ proj_w.T + proj_b

Sharding: data-parallel over batch, one batch per NeuronCore (8 cores).
No collectives needed.

Layout strategy per core (v2):
  - x / qkv_w are cast fp32->bf16 on DVE right after DMA, then PE-transposed
    in bf16 (1 cyc/col vs ~4 for fp32) to xT [C, NTOK], wT [C, 3C].  The
    transpose PSUM tiles are bf16 so the PSUM->SBUF copies run in the DVE
    16-bit packed mode.  (proj_w keeps the fp32 transpose path: its PSUM
    tiles borrow the filler ring, which is fp32.)
  - q,k computed feature-major into a 2-slot rotating buffer (slot =
    pair%2): qk[P, slot, {q,k}, NTOK] (q pre-scaled by 1/8).
  - v computed token-major, augmented per head as [1 | 63 zeros | v]:
    row 0 of the AV output is the softmax denominator.
  - scores computed TRANSPOSED: S.T[tk, tq] = K_h @ Q_h.T.  The two heads
    of a pair sit on PE row groups 0-63 / 64-127, so their score matmuls
    run CONCURRENTLY (tile_position row packing), filling the two halves
    of one [128, 1024] PSUM tile -> ONE exp instruction per (pair, cj, tk)
    covers both heads (N=1024 amortizes the ~300-cycle ACT overhead).
  - search loop is cj-outer (two 512-token query chunks) so each head's
    AV accumulator is one PSUM bank: banks = 4 (scores, double-buffered)
    + 2 (AV accumulators) + 2 (qkv/proj fillers) = 8.
  - normalize fully off the ACT queue: DVE copy PSUM->SBUF, gpsimd
    partition_broadcast of row 0, DVE approx reciprocal, DVE multiply.
  - proj: out[tok, c] = ot_all.T @ pwT; token tiles 2-5 (queries 256-767)
    are emitted as fillers inside the last pair's cj=1 loop (their inputs
    finalize at cj=0 normalize), shrinking the serial tail.

Scheduling: attention paces ACT(exp) and PE about evenly; all qkv / v /
proj-weight work is software-pipelined into the search loops as filler.
All matmuls bf16 (fp32 PSUM accumulation).
"""

import numpy as np

import concourse.bacc as bacc
import concourse.mybir as mybir
import concourse.tile as tile
from concourse.masks import make_identity

P = 128
NTOK = 1280
C = 768
H = 12
HD = 64
NT = 256          # template tokens  [0, NT)
TT = NTOK // P    # 10 token tiles
CT = C // P       # 6 channel tiles
SCALE = HD ** -0.5

F32 = mybir.dt.float32
BF16 = mybir.dt.bfloat16
EXP = mybir.ActivationFunctionType.Exp
MULT = mybir.AluOpType.mult
ADD = mybir.AluOpType.add


def build_nc():
    from contextlib import ExitStack

    nc = bacc.Bacc("TRN2", target_bir_lowering=False, debug=False, num_devices=8)
    x_ext = nc.dram_tensor("x", [NTOK, C], F32, kind="ExternalInput")
    w_ext = nc.dram_tensor("qkv_w", [3 * C, C], F32, kind="ExternalInput")
    pw_ext = nc.dram_tensor("proj_w", [C, C], F32, kind="ExternalInput")
    pb_ext = nc.dram_tensor("proj_b", [1, C], F32, kind="ExternalInput")
    out_ext = nc.dram_tensor("out", [NTOK, C], F32, kind="ExternalOutput")

    with tile.TileContext(nc) as tc, ExitStack() as ctx:
        const = ctx.enter_context(tc.tile_pool(name="const", bufs=1))
        big = ctx.enter_context(tc.tile_pool(name="big", bufs=1))

        identb = const.tile([P, P], BF16)
        make_identity(nc, identb)
        bias_bc = const.tile([P, C], F32)
        bias_row = const.tile([1, C], F32)
        nc.sync.dma_start(bias_row[:], pb_ext.ap())
        nc.gpsimd.partition_broadcast(bias_bc[:], bias_row[0:1, :])

        # transposed operands in XBAR-blocked layout [128, row, ct, 128]:
        # row r, channel tile ct holds (x|w)[r*128:(r+1)*128, ct*128:(ct+1)*128].T
        # so one dma_start_transpose per 128-row input block fills [:, r, :, :]
        xT = big.tile([P, TT, CT, P], BF16)    # x.T   (feature-major x)
        wT = big.tile([P, 18, CT, P], BF16)    # qkv_w.T
        pwT = big.tile([P, CT, CT, P], BF16)   # proj_w.T
        pg = big.tile([P, CT, C], F32)         # proj_w fp32 staging
        pgb = big.tile([P, CT, C], BF16)       # proj_w bf16 (pre-transpose)

        # ---- startup: load, cast to bf16, transpose via the DMA XBAR
        # (dma_start_transpose; 16x128 tiles) -> zero PE transpose work ----
        with tc.tile_pool(name="staging", bufs=2) as staging, \
                tc.tile_pool(name="ps_tp", bufs=2, space="PSUM") as ps_tp:

            # HAM warmup: keep the PE busy during the initial input-DMA wait
            # so its clock gate opens (1.2 -> 2.4 GHz) before the qkv stream.
            warm_ps = ps_tp.tile([P, 1024], BF16, tag="tp")
            for i in range(32):
                nc.tensor.transpose(warm_ps[:, :P], identb[:], identb[:])
            nc.vector.tensor_copy(identb[:], warm_ps[:, :P])

            def emit_xg(g):
                xg = staging.tile([P, CT, C], F32, tag="g", name=f"xg{g}")
                xgb = staging.tile([P, CT, C], BF16, tag="gb", name=f"xgb{g}")
                for j in range(5):
                    t0 = (g * 5 + j) * P
                    nc.sync.dma_start(xg[:, j, :], x_ext.ap()[t0:t0 + P, :])
                    nc.vector.tensor_copy(xgb[:, j, :], xg[:, j, :])
                    nc.sync.dma_start_transpose(
                        xT[:, g * 5 + j, :, :], xgb[:, j, :])

            def emit_wg(g):
                wg = staging.tile([P, CT, C], F32, tag="g", name=f"wg{g}")
                wgb = staging.tile([P, CT, C], BF16, tag="gb", name=f"wgb{g}")
                for j in range(6):
                    f0 = (g * 6 + j) * P
                    nc.sync.dma_start(wg[:, j, :], w_ext.ap()[f0:f0 + P, :])
                    nc.vector.tensor_copy(wgb[:, j, :], wg[:, j, :])
                    nc.sync.dma_start_transpose(
                        wT[:, g * 6 + j, :, :], wgb[:, j, :])

            emit_xg(0)
            emit_wg(0)
            emit_wg(1)
            emit_xg(1)
            emit_wg(2)
            # ---- proj_w: cast + XBAR-transpose too (pure DMA/DVE work,
            # rides the idle queues during the attention phase) ----
            for j in range(CT):
                nc.sync.dma_start(pg[:, j, :], pw_ext.ap()[j * P:(j + 1) * P, :])
                nc.vector.tensor_copy(pgb[:, j, :], pg[:, j, :])
                nc.sync.dma_start_transpose(pwT[:, j, :, :], pgb[:, j, :])

        big2 = ctx.enter_context(tc.tile_pool(name="big2", bufs=1))
        # q (scaled) and k, feature-major, 2-slot rotation keyed by pair%2
        qk = big2.tile([P, 2, 2, NTOK], BF16)
        v_sb = big2.tile([P, TT, H, P], BF16)  # [1 | 63 zeros | v] per head
        ot_all = big2.tile([P, CT, NTOK], BF16)     # attention out, feature-major
        out_sb = big2.tile([P, TT, C], F32)

        # v_aug layout per head: col 0 = ones (softmax denominator row),
        # cols 1:64 = zeros (padding so O lands at partitions 64:128)
        nc.gpsimd.memset(v_sb[:, :, :, 0:64], 0.0)
        nc.gpsimd.memset(v_sb[:, :, :, 0:1], 1.0)

        ps_fill = ctx.enter_context(tc.tile_pool(name="ps_fill", bufs=2, space="PSUM"))

        def transpose_blocks_f32(srcs, dst_full):
            """fp32 transpose path via the filler ring (used only for proj_w,
            36 blocks: not worth a dedicated bf16 staging)."""
            i = 0
            while i < len(srcs):
                n = min(4, len(srcs) - i)
                pt = ps_fill.tile([P, 512], F32, tag="fill")
                for j in range(n):
                    nc.tensor.transpose(
                        pt[:, j * P:(j + 1) * P], srcs[i + j], ident_f()
                    )
                nc.vector.tensor_copy(
                    dst_full[:, i * P:(i + n) * P], pt[:, : n * P]
                )
                i += n

        # fp32 identity for the proj_w transposes (made lazily, as filler)
        _identf = [None]

        def ident_f():
            if _identf[0] is None:
                _identf[0] = const.tile([P, P], F32, name="identf")
                make_identity(nc, _identf[0])
            return _identf[0]

        # ---- qkv projection (emitted interleaved with attention below) ----
        def emit_qk_chunk(hp, which, c0, cw):
            """qk[slot, which] = (q|k) row block of head pair hp,
            feature-major, for token chunk [c0, c0+cw)."""
            ft = hp + 6 * which
            ps = ps_fill.tile([P, 512], F32, tag="fill", name=f"qkp{ft}_{c0}")
            for ct in range(CT):
                nc.tensor.matmul(
                    ps[:, :cw],
                    wT[:, ft, ct, :],
                    xT[:, c0 // P:(c0 + cw) // P, ct, :],
                    start=(ct == 0), stop=(ct == CT - 1),
                )
            if which == 0:  # q: fold in softmax scale
                nc.vector.tensor_scalar_mul(
                    qk[:, hp % 2, 0, c0:c0 + cw], ps[:, :cw], SCALE
                )
            else:
                nc.vector.tensor_copy(qk[:, hp % 2, 1, c0:c0 + cw], ps[:, :cw])

        def qk_pair_chunks(p):
            # q/k interleaved so the chunks a consumer needs first come out
            # adjacent; template needs both c0 chunks only
            return [(p, w, c0, cw)
                    for c0, cw in ((0, 512), (512, 512), (1024, 256))
                    for w in (0, 1)]

        # v token-major: v[tok, f] = x @ qkv_w.T cols [1536, 2304)
        def emit_v_chunk(tt, half):
            c0, cw, h0, nh = ((0, 512, 0, 8), (512, 256, 8, 4))[half]
            ps = ps_fill.tile([P, 512], F32, tag="fill", name=f"vp{tt}_{half}")
            for ct in range(CT):
                nc.tensor.matmul(
                    ps[:, :cw],
                    xT[:, tt, ct, :],
                    wT[:, 12 + c0 // P:12 + (c0 + cw) // P, ct, :],
                    start=(ct == 0), stop=(ct == CT - 1),
                )
            nc.vector.tensor_copy(
                v_sb[:, tt, h0:h0 + nh, 64:128],
                ps[:, :cw].rearrange("p (h e) -> p h e", e=HD),
            )

        # ---- output projection ----
        def emit_proj_chunk(tt, half):
            c0, cw = ((0, 512), (512, 256))[half]
            ps = ps_fill.tile([P, 512], F32, tag="fill", name=f"prj{tt}_{c0}")
            for ct in range(CT):
                nc.tensor.matmul(
                    ps[:, :cw],
                    ot_all[:, ct, tt * P:(tt + 1) * P],
                    pwT[:, c0 // P:(c0 + cw) // P, ct, :],
                    start=(ct == 0), stop=(ct == CT - 1),
                )
            nc.vector.tensor_tensor(
                out_sb[:, tt, c0:c0 + cw], ps[:, :cw],
                bias_bc[:, c0:c0 + cw], ADD,
            )
            if half == 1:
                nc.sync.dma_start(out_ext.ap()[tt * P:(tt + 1) * P, :],
                                  out_sb[:, tt, :])

        def emit_filler(kind, arg):
            if kind == "qk":
                emit_qk_chunk(*arg)
            elif kind == "v":
                emit_v_chunk(*arg)
            elif kind == "proj":
                emit_proj_chunk(*arg)
            else:
                raise AssertionError(kind)

        # q/k for head pair 0 up front
        for a in qk_pair_chunks(0):
            emit_qk_chunk(*a)

        # only the first two token tiles of v are needed before pair 0 starts
        # (template + first search units); the rest stream as pair-0 filler
        for tt in (0, 1):
            emit_v_chunk(tt, 0)
            emit_v_chunk(tt, 1)

        # ---- attention ----
        ps_sc = ctx.enter_context(tc.tile_pool(name="ps_sc", bufs=2, space="PSUM"))
        ps_ot = ctx.enter_context(tc.tile_pool(name="ps_ot", bufs=2, space="PSUM"))
        pts = ctx.enter_context(tc.tile_pool(name="pts", bufs=4))
        dn = ctx.enter_context(tc.tile_pool(name="dn", bufs=2))
        rbp = ctx.enter_context(tc.tile_pool(name="rbp", bufs=2))

        def qh(h, c0, cw):
            b = (h % 2) * 64
            return qk[b:b + 64, (h // 2) % 2, 0, c0:c0 + cw]

        def kh(h, tk):
            b = (h % 2) * 64
            return qk[b:b + 64, (h // 2) % 2, 1, tk * P:(tk + 1) * P]

        def normalize(h, ot_ps, c0, cw):
            """ot_ps: [128, cw] psum (row 0 = denominators, rows 64:128 = O.T
            for tq cols [c0, c0+cw)). Normalize and write to ot_all, fully off
            the ACT queue (one wide DVE copy lifts PSUM->SBUF so the PSUM
            slot frees early)."""
            b = (h % 2) * 64
            den = dn.tile([P, 512], F32, tag="dn")
            nc.vector.tensor_copy(den[:, :cw], ot_ps[:, :cw])
            rb = rbp.tile([P, 512], F32, tag="rb")
            nc.gpsimd.partition_broadcast(rb[:, :cw], den[0:1, :cw])
            # approx reciprocal (~18 bits, plenty for bf16 outputs)
            nc.vector.reciprocal_approx_fast(rb[:, :cw], rb[:, :cw])
            nc.vector.tensor_tensor(
                ot_all[b:b + 64, h // 2, c0:c0 + cw],
                den[64:128, :cw], rb[64:128, :cw], MULT,
            )

        for hp in range(6):
            h0, h1 = 2 * hp, 2 * hp + 1
            # filler work fed into PE idle slots while the ACT-bound
            # attention runs, split across the two cj chunks.
            # pair 0 carries the v token tiles 2..9 (JIT ahead of their AV
            # use in cj 0); pair 4 carries the deferred proj_w transposes;
            # pair 5 carries the early proj tiles (set after cj-0 normalize).
            if hp == 0:
                pend = [[("v", (tt, half)) for tt in range(2, TT)
                         for half in (0, 1)],
                        [("qk", a) for a in qk_pair_chunks(1)]]
            elif hp < 4:
                nxt = [("qk", a) for a in qk_pair_chunks(hp + 1)]
                pend = [nxt[:3], nxt[3:]]
            elif hp == 4:
                nxt = [("qk", a) for a in qk_pair_chunks(5)]
                pend = [nxt[:3], nxt[3:]]
            else:
                pend = [[], []]  # cj1 list filled after cj0 normalize

            # template block, both heads fused: queries [0,256) x keys [0,256)
            st_t = ps_sc.tile([P, 1024], F32, tag="sc", name=f"tst{hp}")
            for tj in range(2):
                for hi, h in enumerate((h0, h1)):
                    nc.tensor.matmul(
                        st_t[:, hi * 512 + tj * NT: hi * 512 + (tj + 1) * NT],
                        kh(h, tj), qh(h, 0, NT), start=True, stop=True,
                    )
            pt_t = pts.tile([P, 1024], BF16, tag="pt", name=f"tpt{hp}")
            nc.scalar.activation(pt_t[:], st_t[:], EXP)
            for hi, h in enumerate((h0, h1)):
                to = ps_fill.tile([P, 512], F32, tag="fill", name=f"to{h}")
                for tj in range(2):
                    nc.tensor.matmul(
                        to[:, :NT], v_sb[:, tj, h, :],
                        pt_t[:, hi * 512 + tj * NT: hi * 512 + (tj + 1) * NT],
                        start=(tj == 0), stop=(tj == 1),
                    )
                normalize(h, to, 0, NT)

            # search: queries [256, 1280) attend all keys, cj-outer
            for cj in range(2):
                c0 = NT + cj * 512
                pending = pend[cj]
                ots = {h: ps_ot.tile([P, 512], F32, tag="ot",
                                     name=f"ot{h}_{cj}")
                       for h in (h0, h1)}
                for tk in range(TT):
                    st = ps_sc.tile([P, 1024], F32, tag="sc",
                                    name=f"st{hp}_{cj}_{tk}")
                    # the two heads run CONCURRENTLY on PE row groups
                    # 0-63 / 64-127, filling the two halves of one tile
                    for hi, h in enumerate((h0, h1)):
                        nc.tensor.matmul(
                            st[:, hi * 512:(hi + 1) * 512],
                            kh(h, tk), qh(h, c0, 512), start=True, stop=True,
                        )
                    pt = pts.tile([P, 1024], BF16, tag="pt",
                                  name=f"pt{hp}_{cj}_{tk}")
                    nc.scalar.activation(pt[:], st[:], EXP)
                    for hi, h in enumerate((h0, h1)):
                        nc.tensor.matmul(
                            ots[h][:, :], v_sb[:, tk, h, :],
                            pt[:, hi * 512:(hi + 1) * 512],
                            start=(tk == 0), stop=(tk == TT - 1),
                        )
                    # feed filler into the PE stream (pair 0 cj 0 carries the
                    # v tail and needs a higher drain rate to stay JIT-ahead
                    # of its AV consumers)
                    for _ in range(2 if (hp == 0 and cj == 0) else 1):
                        if pending:
                            emit_filler(*pending.pop(0))
                for h in (h0, h1):
                    normalize(h, ots[h], c0, 512)
                while pending:
                    emit_filler(*pending.pop(0))
                if hp == 5 and cj == 0:
                    # queries 256-767 (token tiles 2-5) are final once every
                    # pair's cj-0 normalize is done -> their proj overlaps
                    # the cj-1 attention as filler
                    pend[1] = [("proj", (tt, half)) for tt in (2, 3, 4, 5)
                               for half in (0, 1)]

        # remaining output projection (template tiles + cj-1 tiles)
        for tt in (0, 1, 6, 7, 8, 9):
            for half in (0, 1):
                emit_proj_chunk(tt, half)

    nc.compile()
    return nc


_NC = None


def _get_nc():
    global _NC
    if _NC is None:
        _NC = build_nc()
    return _NC


def kernel(x, qkv_w, proj_w, proj_b, **_ignored):
    from concourse.bass_utils import run_bass_kernel_spmd

    x = np.ascontiguousarray(np.asarray(x), dtype=np.float32)
    qkv_w = np.ascontiguousarray(np.asarray(qkv_w), dtype=np.float32)
    proj_w = np.ascontiguousarray(np.asarray(proj_w), dtype=np.float32)
    proj_b = np.ascontiguousarray(np.asarray(proj_b), dtype=np.float32).reshape(1, C)

    nc = _get_nc()
    in_maps = [
        {"x": x[i], "qkv_w": qkv_w, "proj_w": proj_w, "proj_b": proj_b}
        for i in range(8)
    ]
    res = run_bass_kernel_spmd(nc, in_maps, list(range(8)))
    return np.stack([res.results[i]["out"] for i in range(8)])


if __name__ == "__main__":
    rng = np.random.default_rng(0)
    ins = {
        "x": rng.standard_normal((8, NTOK, C), dtype=np.float32),
        "qkv_w": rng.standard_normal((3 * C, C), dtype=np.float32) * 0.02,
        "proj_w": rng.standard_normal((C, C), dtype=np.float32) * 0.02,
        "proj_b": np.zeros(C, dtype=np.float32),
    }
    out = kernel(**ins)
    print("out", out.shape, out.dtype)
